# revision 23
# baseline (speedup 1.0000x reference)
"""Trainium2 Bass kernel for CompoundWordAutoregressiveWrapper loss_fn.

Computes 8 scalar losses:
  - 7 masked-mean cross-entropy losses, one per projection head
    ([2,1024,6913] logits each), target channels 0..6 of x[:,1:,:],
    mask = (x[:,1:,0] != 0).
  - 1 masked-mean MSE between a constant f0 (the "temps" branch of the
    reference constant-folds: softmax over an axis of size 1 is
    identically 1.0, so f = colsums(trig_table)/6*6 is input-independent)
    and x[:,1:,11].

Strategy (data-parallel, per sharding hint): flatten p = B*S = 2048 rows,
shard 256 rows to each of 8 NeuronCores. Each core:
  - streams its 7x[256,6913] logit slices from HBM once (memory-bound),
    each 128-row tile split into two half-loads issued on the two HWDGE
    rings (SP + ACT) so both rings advance the same tile;
  - ScalarE activation(Exp, accum_out) produces per-row sum(exp(logits));
  - logits[row, target[row]] is fetched by indirect (gather) DMA straight
    from DRAM via SWDGE using host-precomputed flat element offsets -- no
    vector-engine work in the steady state;
  - nll = log(sumexp) - picked; masked sums are reduced across partitions
    with one PE matmul against a ones vector.
Host combines the 8 cores' partial numerators/denominators and divides.
"""

import sys

if "/opt/trn_rl_repo" not in sys.path:
    sys.path.insert(0, "/opt/trn_rl_repo")

import numpy as np

_B, _S = 2, 1024
_P = _B * _S  # 2048 flattened rows
_V = 6913
_NCORES = 8
_ROWS = _P // _NCORES  # 256 rows per core
_HEADS = (
    "proj_type",
    "proj_barbeat",
    "proj_tempo",
    "proj_instrument",
    "proj_note_name",
    "proj_octave",
    "proj_duration",
)
_NHEADS = len(_HEADS)

# f = (s @ d)/6 with s identically 6.0 -> f[...,0] = column sum of
# sin(1*ang) over the 6912-entry trig table; mathematically ~0, fp
# residual ~1.6e-5 (impact on the MSE is ~4e-8 relative).
_F0 = 1.6023243915697094e-05

_PROGRAM_CACHE = {}


def _build(rows=_ROWS, v=_V):
    """Build the SPMD Bass program for one core: rows x v per head."""
    import concourse.bass as bass
    import concourse.mybir as mybir
    from concourse import bacc, tile

    f32 = mybir.dt.float32
    i32 = mybir.dt.int32
    AF = mybir.ActivationFunctionType
    ALU = mybir.AluOpType

    assert rows % 128 == 0
    ntiles = rows // 128
    ncols = ntiles * _NHEADS  # one column per (row-tile, head)
    nout = ncols + 2 * ntiles  # + mask counts + mse partials
    assert nout <= 128
    vh = v // 2  # half-tile split point

    # Bacc (not plain Bass): its compile() legalizes multi-wait sync via
    # InstEventSemaphore -- TRN2 compute instructions encode at most 1 wait.
    nc = bacc.Bacc(trn_type="TRN2")
    # 1-D logits tensors: the flat view is what the gather DMA indexes into;
    # the streaming loads re-view them as [rows, v].
    lg_dram = [
        nc.dram_tensor(f"lg{h}", [rows * v], f32, kind="ExternalInput")
        for h in range(_NHEADS)
    ]
    # aux columns: 0 mask, 1 x[...,11], rest pad
    aux_dram = nc.dram_tensor("aux", [rows, 4], f32, kind="ExternalInput")
    # goff[r, h] = r*v + target[r, h]: flat element offsets for the gather
    goff_dram = nc.dram_tensor("goff", [rows, 8], i32, kind="ExternalInput")
    out_dram = nc.dram_tensor("out", [nout, 1], f32, kind="ExternalOutput")

    lg2d = [d.rearrange("(r c) -> r c", c=v) for d in lg_dram]
    # [N, 1] view for the gather: offsets index axis 0, one element each
    lgflat = [d.rearrange("(n o) -> n o", o=1) for d in lg_dram]

    with tile.TileContext(nc) as tc:
        with (
            tc.tile_pool(name="lg", bufs=6) as lgp,
            tc.tile_pool(name="es", bufs=1) as esp,
            tc.tile_pool(name="sm", bufs=1) as smp,
            tc.tile_pool(name="ps", bufs=1, space=bass.MemorySpace.PSUM) as psp,
        ):
            # small loads on SWDGE so the HWDGE rings start with the big
            # streaming loads
            aux = []
            goff = []
            for t in range(ntiles):
                a = smp.tile([128, 4], f32, tag=f"aux{t}")
                nc.gpsimd.dma_start(a[:], aux_dram[t * 128 : (t + 1) * 128, :])
                aux.append(a)
                g = smp.tile([128, 8], i32, tag=f"goff{t}")
                nc.gpsimd.dma_start(g[:], goff_dram[t * 128 : (t + 1) * 128, :])
                goff.append(g)
            sumexp = smp.tile([128, ncols], f32, tag="sumexp")
            picked = smp.tile([128, ncols], f32, tag="picked")
            logsum = smp.tile([128, ncols], f32, tag="logsum")
            nll = smp.tile([128, ncols], f32, tag="nll")
            stats = smp.tile([128, nout], f32, tag="stats")
            ones = smp.tile([128, 1], f32, tag="ones")
            tmp2 = smp.tile([128, ntiles], f32, tag="tmp2")
            outs = smp.tile([nout, 1], f32, tag="outs")
            nc.vector.memset(ones[:], 1.0)

            for h in range(_NHEADS):
                for t in range(ntiles):
                    col = t * _NHEADS + h
                    lg = lgp.tile([128, v], f32, tag="lg")
                    # each tile as two half-loads, one per HWDGE ring, so
                    # both rings advance the same tile in lock-step
                    src = lg2d[h][t * 128 : (t + 1) * 128, :]
                    nc.sync.dma_start(lg[:, 0:vh], src[:, 0:vh])
                    nc.scalar.dma_start(lg[:, vh:v], src[:, vh:v])
                    es = esp.tile([128, v], f32, tag="es")
                    nc.scalar.activation(
                        es[:], lg[:], AF.Exp, accum_out=sumexp[:, col : col + 1]
                    )

            # gather DMAs: one per (head, row-tile), indexing DRAM directly.
            # Emitted after the streaming loads so the SWDGE traffic doesn't
            # steal SDMA slices from the first tiles; results are only
            # consumed by the nll subtract at the end.
            for h in range(_NHEADS):
                for t in range(ntiles):
                    col = t * _NHEADS + h
                    nc.gpsimd.indirect_dma_start(
                        out=picked[:, col : col + 1],
                        out_offset=None,
                        in_=lgflat[h][:],
                        in_offset=bass.IndirectOffsetOnAxis(
                            ap=goff[t][:, h : h + 1], axis=0
                        ),
                    )

            nc.scalar.activation(logsum[:], sumexp[:], AF.Ln)
            nc.vector.tensor_tensor(nll[:], logsum[:], picked[:], op=ALU.subtract)
            for t in range(ntiles):
                mask_ap = aux[t][:, 0:1]
                lo = t * _NHEADS
                nc.vector.tensor_scalar_mul(
                    stats[:, lo : lo + _NHEADS], nll[:, lo : lo + _NHEADS], mask_ap
                )
                nc.vector.tensor_copy(stats[:, ncols + t : ncols + t + 1], mask_ap)
                # mask*(t11-f0) then square it (mask is 0/1 so mask^2 == mask)
                nc.vector.tensor_scalar(
                    tmp2[:, t : t + 1],
                    aux[t][:, 1:2],
                    float(_F0),
                    mask_ap,
                    op0=ALU.subtract,
                    op1=ALU.mult,
                )
                nc.vector.tensor_tensor(
                    stats[:, ncols + ntiles + t : ncols + ntiles + t + 1],
                    tmp2[:, t : t + 1],
                    tmp2[:, t : t + 1],
                    op=ALU.mult,
                )

            pt = psp.tile([nout, 1], f32, tag="ps")
            nc.tensor.matmul(pt[:], stats[:], ones[:], start=True, stop=True)
            nc.vector.tensor_copy(outs[:], pt[:])
            nc.sync.dma_start(out_dram[:], outs[:])

    return nc


def _get_program():
    if "nc" not in _PROGRAM_CACHE:
        nc = _build()
        nc.finalize()
        _PROGRAM_CACHE["nc"] = nc
    return _PROGRAM_CACHE["nc"]


def _make_in_maps(inputs):
    x = np.asarray(inputs["x"])
    heads = [
        np.ascontiguousarray(np.asarray(inputs[n], dtype=np.float32)).reshape(_P * _V)
        for n in _HEADS
    ]
    tgt = x[:, 1:, :].reshape(_P, 12)
    aux = np.zeros((_P, 4), np.float32)
    aux[:, 0] = (tgt[:, 0] != 0).astype(np.float32)
    aux[:, 1] = tgt[:, 11].astype(np.float32)
    goff = np.zeros((_P, 8), np.int32)
    rloc = (np.arange(_P, dtype=np.int64) % _ROWS) * _V
    for h in range(_NHEADS):
        goff[:, h] = (rloc + tgt[:, h].astype(np.int64)).astype(np.int32)
    in_maps = []
    for c in range(_NCORES):
        sl = slice(c * _ROWS, (c + 1) * _ROWS)
        fl = slice(c * _ROWS * _V, (c + 1) * _ROWS * _V)
        m = {f"lg{h}": heads[h][fl] for h in range(_NHEADS)}
        m["aux"] = aux[sl]
        m["goff"] = goff[sl]
        in_maps.append(m)
    return in_maps


def _combine(core_outs):
    """core_outs: [ncores, nout] partial sums -> [8] float32 losses."""
    o = np.asarray(core_outs, dtype=np.float64)
    ntiles = _ROWS // 128
    ncols = ntiles * _NHEADS
    tot = o[:, ncols : ncols + ntiles].sum()
    if tot == 0.0:
        return np.zeros(8, np.float32)
    ce = np.zeros(_NHEADS, np.float64)
    for t in range(ntiles):
        ce += o[:, t * _NHEADS : (t + 1) * _NHEADS].sum(axis=0)
    mse = o[:, ncols + ntiles : ncols + 2 * ntiles].sum()
    return np.concatenate([ce / tot, [mse / tot]]).astype(np.float32)


def _execute(inputs, trace=False, **kwargs):
    from concourse import bass_utils

    nc = _get_program()
    in_maps = _make_in_maps(inputs)
    res = bass_utils.run_bass_kernel_spmd(
        nc, in_maps, core_ids=list(range(_NCORES)), trace=trace, **kwargs
    )
    core_outs = np.stack([np.asarray(r["out"])[:, 0] for r in res.results])
    return _combine(core_outs), res


def kernel(**inputs) -> np.ndarray:
    out, _ = _execute(inputs)
    return out


# revision 26
# speedup vs baseline: 1.0207x; 1.0207x over previous
"""Trainium2 Bass kernel for CompoundWordAutoregressiveWrapper loss_fn.

Computes 8 scalar losses:
  - 7 masked-mean cross-entropy losses, one per projection head
    ([2,1024,6913] logits each), target channels 0..6 of x[:,1:,:],
    mask = (x[:,1:,0] != 0).
  - 1 masked-mean MSE between a constant f0 (the "temps" branch of the
    reference constant-folds: softmax over an axis of size 1 is
    identically 1.0, so f = colsums(trig_table)/6*6 is input-independent)
    and x[:,1:,11].

Strategy (data-parallel, per sharding hint): flatten p = B*S = 2048 rows,
shard 256 rows to each of 8 NeuronCores. Each core:
  - streams its 7x[256,6913] logit slices from HBM once (memory-bound),
    each 128-row tile split into two half-loads issued on the two HWDGE
    rings (SP + ACT) so both rings advance the same tile;
  - ScalarE activation(Exp, accum_out) produces per-row sum(exp(logits));
  - logits[row, target[row]] is fetched by indirect (gather) DMA straight
    from DRAM via SWDGE using host-precomputed flat element offsets -- no
    vector-engine work in the steady state;
  - nll = log(sumexp) - picked; masked sums are reduced across partitions
    with one PE matmul against a ones vector.
Host combines the 8 cores' partial numerators/denominators and divides.
"""

import sys

if "/opt/trn_rl_repo" not in sys.path:
    sys.path.insert(0, "/opt/trn_rl_repo")

import numpy as np

_B, _S = 2, 1024
_P = _B * _S  # 2048 flattened rows
_V = 6913
_NCORES = 8
_ROWS = _P // _NCORES  # 256 rows per core
_HEADS = (
    "proj_type",
    "proj_barbeat",
    "proj_tempo",
    "proj_instrument",
    "proj_note_name",
    "proj_octave",
    "proj_duration",
)
_NHEADS = len(_HEADS)

# f = (s @ d)/6 with s identically 6.0 -> f[...,0] = column sum of
# sin(1*ang) over the 6912-entry trig table; mathematically ~0, fp
# residual ~1.6e-5 (impact on the MSE is ~4e-8 relative).
_F0 = 1.6023243915697094e-05

_PROGRAM_CACHE = {}


def _build(rows=_ROWS, v=_V):
    """Build the SPMD Bass program for one core: rows x v per head."""
    import concourse.bass as bass
    import concourse.mybir as mybir
    from concourse import bacc, tile

    f32 = mybir.dt.float32
    i32 = mybir.dt.int32
    AF = mybir.ActivationFunctionType
    ALU = mybir.AluOpType

    assert rows % 128 == 0
    ntiles = rows // 128
    ncols = ntiles * _NHEADS  # one column per (row-tile, head)
    nout = ncols + 2 * ntiles  # + mask counts + mse partials
    assert nout <= 128
    vh = v // 2  # half-tile split point

    # Bacc (not plain Bass): its compile() legalizes multi-wait sync via
    # InstEventSemaphore -- TRN2 compute instructions encode at most 1 wait.
    nc = bacc.Bacc(trn_type="TRN2")
    # 1-D logits tensors: the flat view is what the gather DMA indexes into;
    # the streaming loads re-view them as [rows, v].
    lg_dram = [
        nc.dram_tensor(f"lg{h}", [rows * v], f32, kind="ExternalInput")
        for h in range(_NHEADS)
    ]
    # aux columns: 0 mask, 1 x[...,11], rest pad
    aux_dram = nc.dram_tensor("aux", [rows, 4], f32, kind="ExternalInput")
    # goff[r, h] = r*v + target[r, h]: flat element offsets for the gather
    goff_dram = nc.dram_tensor("goff", [rows, 8], i32, kind="ExternalInput")
    out_dram = nc.dram_tensor("out", [nout, 1], f32, kind="ExternalOutput")

    lg2d = [d.rearrange("(r c) -> r c", c=v) for d in lg_dram]
    # [N, 1] view for the gather: offsets index axis 0, one element each
    lgflat = [d.rearrange("(n o) -> n o", o=1) for d in lg_dram]

    with tile.TileContext(nc) as tc:
        with (
            tc.tile_pool(name="lg", bufs=6) as lgp,
            tc.tile_pool(name="es", bufs=1) as esp,
            tc.tile_pool(name="sm", bufs=1) as smp,
            tc.tile_pool(name="ps", bufs=1, space=bass.MemorySpace.PSUM) as psp,
        ):
            # small loads on SWDGE so the HWDGE rings start with the big
            # streaming loads
            aux = []
            goff = []
            for t in range(ntiles):
                a = smp.tile([128, 4], f32, tag=f"aux{t}")
                nc.gpsimd.dma_start(a[:], aux_dram[t * 128 : (t + 1) * 128, :])
                aux.append(a)
                g = smp.tile([128, 8], i32, tag=f"goff{t}")
                nc.gpsimd.dma_start(g[:], goff_dram[t * 128 : (t + 1) * 128, :])
                goff.append(g)
            # per-half exp sums: cols [0:ncols] = first halves, [ncols:2*ncols]
            # = second halves; summed pairwise into `sumexp` before the log
            sumexp2 = smp.tile([128, 2 * ncols], f32, tag="sumexp2")
            sumexp = smp.tile([128, ncols], f32, tag="sumexp")
            picked = smp.tile([128, ncols], f32, tag="picked")
            logsum = smp.tile([128, ncols], f32, tag="logsum")
            nll = smp.tile([128, ncols], f32, tag="nll")
            stats = smp.tile([128, nout], f32, tag="stats")
            ones = smp.tile([128, 1], f32, tag="ones")
            tmp2 = smp.tile([128, ntiles], f32, tag="tmp2")
            outs = smp.tile([nout, 1], f32, tag="outs")
            nc.vector.memset(ones[:], 1.0)

            for h in range(_NHEADS):
                for t in range(ntiles):
                    col = t * _NHEADS + h
                    lg = lgp.tile([128, v], f32, tag="lg")
                    # each tile as two half-loads, one per HWDGE ring, so
                    # both rings advance the same tile in lock-step; each
                    # half gets its own exp pass as soon as it lands (the
                    # exp output is never read, so write it as bf16)
                    src = lg2d[h][t * 128 : (t + 1) * 128, :]
                    nc.sync.dma_start(lg[:, 0:vh], src[:, 0:vh])
                    nc.scalar.dma_start(lg[:, vh:v], src[:, vh:v])
                    es = esp.tile([128, v], mybir.dt.bfloat16, tag="es")
                    nc.scalar.activation(
                        es[:, 0:vh],
                        lg[:, 0:vh],
                        AF.Exp,
                        accum_out=sumexp2[:, col : col + 1],
                    )
                    nc.scalar.activation(
                        es[:, vh:v],
                        lg[:, vh:v],
                        AF.Exp,
                        accum_out=sumexp2[:, ncols + col : ncols + col + 1],
                    )

            # gather DMAs: one per (head, row-tile), indexing DRAM directly.
            # Emitted after the streaming loads so the SWDGE traffic doesn't
            # steal SDMA slices from the first tiles; results are only
            # consumed by the nll subtract at the end.
            for h in range(_NHEADS):
                for t in range(ntiles):
                    col = t * _NHEADS + h
                    nc.gpsimd.indirect_dma_start(
                        out=picked[:, col : col + 1],
                        out_offset=None,
                        in_=lgflat[h][:],
                        in_offset=bass.IndirectOffsetOnAxis(
                            ap=goff[t][:, h : h + 1], axis=0
                        ),
                    )

            nc.vector.tensor_tensor(
                sumexp[:], sumexp2[:, 0:ncols], sumexp2[:, ncols : 2 * ncols],
                op=ALU.add,
            )
            nc.scalar.activation(logsum[:], sumexp[:], AF.Ln)
            nc.vector.tensor_tensor(nll[:], logsum[:], picked[:], op=ALU.subtract)
            for t in range(ntiles):
                mask_ap = aux[t][:, 0:1]
                lo = t * _NHEADS
                nc.vector.tensor_scalar_mul(
                    stats[:, lo : lo + _NHEADS], nll[:, lo : lo + _NHEADS], mask_ap
                )
                nc.vector.tensor_copy(stats[:, ncols + t : ncols + t + 1], mask_ap)
                # mask*(t11-f0) then square it (mask is 0/1 so mask^2 == mask)
                nc.vector.tensor_scalar(
                    tmp2[:, t : t + 1],
                    aux[t][:, 1:2],
                    float(_F0),
                    mask_ap,
                    op0=ALU.subtract,
                    op1=ALU.mult,
                )
                nc.vector.tensor_tensor(
                    stats[:, ncols + ntiles + t : ncols + ntiles + t + 1],
                    tmp2[:, t : t + 1],
                    tmp2[:, t : t + 1],
                    op=ALU.mult,
                )

            pt = psp.tile([nout, 1], f32, tag="ps")
            nc.tensor.matmul(pt[:], stats[:], ones[:], start=True, stop=True)
            nc.vector.tensor_copy(outs[:], pt[:])
            nc.sync.dma_start(out_dram[:], outs[:])

    return nc


def _get_program():
    if "nc" not in _PROGRAM_CACHE:
        nc = _build()
        nc.finalize()
        _PROGRAM_CACHE["nc"] = nc
    return _PROGRAM_CACHE["nc"]


def _make_in_maps(inputs):
    x = np.asarray(inputs["x"])
    heads = [
        np.ascontiguousarray(np.asarray(inputs[n], dtype=np.float32)).reshape(_P * _V)
        for n in _HEADS
    ]
    tgt = x[:, 1:, :].reshape(_P, 12)
    aux = np.zeros((_P, 4), np.float32)
    aux[:, 0] = (tgt[:, 0] != 0).astype(np.float32)
    aux[:, 1] = tgt[:, 11].astype(np.float32)
    goff = np.zeros((_P, 8), np.int32)
    rloc = (np.arange(_P, dtype=np.int64) % _ROWS) * _V
    for h in range(_NHEADS):
        goff[:, h] = (rloc + tgt[:, h].astype(np.int64)).astype(np.int32)
    in_maps = []
    for c in range(_NCORES):
        sl = slice(c * _ROWS, (c + 1) * _ROWS)
        fl = slice(c * _ROWS * _V, (c + 1) * _ROWS * _V)
        m = {f"lg{h}": heads[h][fl] for h in range(_NHEADS)}
        m["aux"] = aux[sl]
        m["goff"] = goff[sl]
        in_maps.append(m)
    return in_maps


def _combine(core_outs):
    """core_outs: [ncores, nout] partial sums -> [8] float32 losses."""
    o = np.asarray(core_outs, dtype=np.float64)
    ntiles = _ROWS // 128
    ncols = ntiles * _NHEADS
    tot = o[:, ncols : ncols + ntiles].sum()
    if tot == 0.0:
        return np.zeros(8, np.float32)
    ce = np.zeros(_NHEADS, np.float64)
    for t in range(ntiles):
        ce += o[:, t * _NHEADS : (t + 1) * _NHEADS].sum(axis=0)
    mse = o[:, ncols + ntiles : ncols + 2 * ntiles].sum()
    return np.concatenate([ce / tot, [mse / tot]]).astype(np.float32)


def _execute(inputs, trace=False, **kwargs):
    from concourse import bass_utils

    nc = _get_program()
    in_maps = _make_in_maps(inputs)
    res = bass_utils.run_bass_kernel_spmd(
        nc, in_maps, core_ids=list(range(_NCORES)), trace=trace, **kwargs
    )
    core_outs = np.stack([np.asarray(r["out"])[:, 0] for r in res.results])
    return _combine(core_outs), res


def kernel(**inputs) -> np.ndarray:
    out, _ = _execute(inputs)
    return out
